# revision 1
# baseline (speedup 1.0000x reference)
"""Grouped 3x3 SAME conv on 8 Trainium2 NeuronCores.

Problem: x[16,56,56,256] NHWC, 8 groups of 32->64 channels, 3x3 SAME,
out[16,56,56,512], fp32.

Strategy (hardcoded):
  - Data-parallel over batch: core i handles images [2i, 2i+1].
  - Host-side layout prep (part of the sharding step): transpose x to
    channels-major, zero-pad spatial to 58x58, pre-replicate the three
    kh-shifted copies, and cast to fp16 (11-bit mantissa; conv accumulates
    in fp32 PSUM, so rel err stays ~5e-4). Device output comes back
    channels-major fp32 and the host transposes back to NHWC.
  - On device: conv = matmuls with contraction stacked over (kh, c) = 96
    partitions; the kw shift is a +-1 column offset on the same SBUF tile.
    Two groups are packed per wave via tile_position col-groups (0,0) and
    (0,64) writing one PSUM [128, N] tile; fp16 streams 1 cycle/row;
    spatial tiles are 8 image rows (N=464, one PSUM bank per matmul).
    Bias is added by DVE during the PSUM->SBUF copy.
"""

import numpy as np

G = 8        # groups
P = 32       # in-channels per group
F = 64       # out-channels per group
H = W = 56
HP = WP = 58           # zero-padded spatial
SP = HP * WP           # 3364 padded pixels
SHIFT = WP             # column shift of one image row
N_CORES = 8
B_PER_CORE = 2
NPAIR = G // 2         # group pairs packed per wave
# spatial tiles over padded cols [58, 3306): 8 image rows each
# (N=464 <= 512: a matmul writes one PSUM bank)
TILES = [((1 + 8 * t) * SHIFT, 8 * SHIFT) for t in range(7)]

_PROG_CACHE = {}


def _build_program():
    import concourse.bacc as bacc
    import concourse.mybir as mybir
    import concourse.tile as tile

    dt = mybir.dt
    nc = bacc.Bacc(
        "TRN2",
        target_bir_lowering=False,
        debug=False,
        num_devices=N_CORES,
    )

    f32 = dt.float32
    f16 = dt.float16

    xT = nc.dram_tensor("xT", [B_PER_CORE, G, 3 * P, SP], f16,
                        kind="ExternalInput")
    wT = nc.dram_tensor("wT", [3 * P, G * 3 * F], f16,
                        kind="ExternalInput")
    bT = nc.dram_tensor("bT", [2 * F, NPAIR], f32, kind="ExternalInput")
    outT = nc.dram_tensor("outT", [B_PER_CORE, G * F, SP], f32,
                          kind="ExternalOutput")

    with tile.TileContext(nc) as tc:
        with (
            tc.tile_pool(name="const", bufs=1) as cpool,
            tc.tile_pool(name="xg", bufs=4) as xpool,
            tc.tile_pool(name="ot", bufs=4) as opool,
            tc.tile_pool(name="ps", bufs=4, space="PSUM") as ppool,
        ):
            wsb = cpool.tile([3 * P, G * 3 * F], f16)
            nc.sync.dma_start(wsb[:], wT[:])
            bsb = cpool.tile([2 * F, NPAIR], f32)
            nc.sync.dma_start(bsb[:], bT[:])

            for b in range(B_PER_CORE):
                for gp in range(NPAIR):
                    ga, gb = 2 * gp, 2 * gp + 1
                    # per group: [96, SP] = 3 kh-shifted replicas of the
                    # group's [32, SP] channel block (host pre-replicated)
                    xa = xpool.tile([3 * P, SP], f16, tag="xa")
                    xb = xpool.tile([3 * P, SP], f16, tag="xb")
                    nc.sync.dma_start(xa[:], xT[b, ga, :, :])
                    nc.sync.dma_start(xb[:], xT[b, gb, :, :])

                    for s, nt in TILES:
                        ps = ppool.tile([2 * F, 8 * SHIFT], f32)
                        for dw in range(3):
                            nc.tensor.matmul(
                                ps[0:F, :nt],
                                wsb[:, (ga * 3 + dw) * F:(ga * 3 + dw + 1) * F],
                                xa[:, s - 1 + dw:s - 1 + dw + nt],
                                start=(dw == 0),
                                stop=(dw == 2),
                                tile_position=(0, 0),
                            )
                            nc.tensor.matmul(
                                ps[F:2 * F, :nt],
                                wsb[:, (gb * 3 + dw) * F:(gb * 3 + dw + 1) * F],
                                xb[:, s - 1 + dw:s - 1 + dw + nt],
                                start=(dw == 0),
                                stop=(dw == 2),
                                tile_position=(0, F),
                            )
                        ot = opool.tile([2 * F, 8 * SHIFT], f32)
                        nc.vector.tensor_scalar_add(ot[:, :nt], ps[:, :nt],
                                                    bsb[:, gp:gp + 1])
                        nc.sync.dma_start(
                            outT[b, gp * 2 * F:(gp + 1) * 2 * F, s:s + nt],
                            ot[:, :nt])

    nc.compile()
    return nc


def _get_program():
    if "nc" not in _PROG_CACHE:
        _PROG_CACHE["nc"] = _build_program()
    return _PROG_CACHE["nc"]


def prepare_in_maps(x, kernels, bias):
    x = np.ascontiguousarray(x, dtype=np.float32)
    kernels = np.ascontiguousarray(kernels, dtype=np.float32)
    bias = np.ascontiguousarray(bias, dtype=np.float32)

    nb = x.shape[0]
    # zero-padded channels-major view of x: [b, g, c, hp*wp], fp16
    xpad = np.zeros((nb, G, P, HP, WP), np.float16)
    xpad[:, :, :, 1:1 + H, 1:1 + W] = (
        x.transpose(0, 3, 1, 2).reshape(nb, G, P, H, W).astype(np.float16)
    )
    xpad = xpad.reshape(nb, G, P, SP)
    # pre-replicated kh-shifted blocks: xT[b,g,32j+c,m] = xpad[...,m+58(j-1)]
    xT = np.zeros((nb, G, 3, P, SP), np.float16)
    xT[:, :, 0, :, SHIFT:] = xpad[:, :, :, :SP - SHIFT]
    xT[:, :, 1, :, :] = xpad
    xT[:, :, 2, :, :SP - SHIFT] = xpad[:, :, :, SHIFT:]
    xT = xT.reshape(nb, G, 3 * P, SP)
    # [kh*c, g*kw*f] weight layout: lhsT slices [96, 64] per (g, kw)
    wT = np.ascontiguousarray(
        kernels.transpose(1, 3, 0, 2, 4).reshape(3 * P, G * 3 * F)
    ).astype(np.float16)
    bT = np.ascontiguousarray(bias.reshape(NPAIR, 2 * F).T)

    return [
        {"xT": np.ascontiguousarray(xT[i * B_PER_CORE:(i + 1) * B_PER_CORE]),
         "wT": wT, "bT": bT}
        for i in range(N_CORES)
    ]


def gather_output(results, nb):
    out = np.empty((nb, H, W, G * F), np.float32)
    for i in range(N_CORES):
        o = results[i]["outT"].reshape(B_PER_CORE, G * F, HP, WP)
        o = o[:, :, 1:1 + H, 1:1 + W]               # drop padded rows/cols
        out[i * B_PER_CORE:(i + 1) * B_PER_CORE] = o.transpose(0, 2, 3, 1)
    return out


def kernel(x, kernels, bias):
    from concourse.bass_utils import run_bass_kernel_spmd

    nc = _get_program()
    in_maps = prepare_in_maps(x, kernels, bias)
    res = run_bass_kernel_spmd(nc, in_maps, list(range(N_CORES)))
    return gather_output(res.results, np.asarray(x).shape[0])



# revision 3
# speedup vs baseline: 1.4033x; 1.4033x over previous
"""Grouped 3x3 SAME conv on 8 Trainium2 NeuronCores.

Problem: x[16,56,56,256] NHWC, 8 groups of 32->64 channels, 3x3 SAME,
out[16,56,56,512], fp32.

Strategy (hardcoded):
  - Data-parallel over batch: core i handles images [2i, 2i+1].
  - Host-side prep: channels-major, zero-pad spatial to 58x58, flatten to
    3364 cols (+1 zero col each side -> 3366), fp16. NO tap replication.
  - On device: the 128x128 PE array is 16 independent 32x32 sub-arrays.
    A group's 32->64 matmul occupies a (32-row x 64-col) region; 8 groups
    tile the full array exactly. Per tap (kh,kw) the 8 groups' matmuls
    are issued back-to-back and stream concurrently (disjoint regions);
    the tap shift is a column offset into the flat padded image. 9 taps
    accumulate into PSUM (per-element has_written -> add). Full PE
    utilization vs 37.5% for the K=96/M=64 formulation.
  - PSUM: 8 banks = 2 images x 4 banks ([128,464] each, two groups per
    bank). DVE adds bias during PSUM->SBUF copy (fp16 out), DMA to HBM.
  - x is DMA'd in 7 overlapping column chunks per (image, channel-half)
    so tile t's matmuls only wait on chunk t, not the whole image.
"""

import numpy as np

G = 8        # groups
P = 32       # in-channels per group
F = 64       # out-channels per group
H = W = 56
HP = WP = 58           # zero-padded spatial
SP = HP * WP           # 3364 padded pixels
XW = SP + 2            # flat padded width (+1 zero col each side)
N_CORES = 8
B_PER_CORE = 2
NT = 7                 # spatial tiles (8 image rows each)
NTW = 8 * WP           # 464 output cols per tile (<=512: one PSUM bank)
CHW = NTW + 2 * (WP + 1)   # 582: chunk width incl tap halo
OW = NT * NTW          # 3248 stored cols per image (rows 1..56)

# group -> (row strip, col half == psum half, bank); the 16 (row,colhalf)
# strips cover the array exactly: left col-half by strip = [0,5,2,7],
# right = [4,1,6,3]; banks: 0=[g0|g1] 1=[g2|g3] 2=[g5|g4] 3=[g7|g6]
ROWSTRIP = [0, 1, 2, 3, 0, 1, 2, 3]      # == g % 4
COLHALF = [0, 1, 0, 1, 1, 0, 1, 0]
BANK = [0, 0, 1, 1, 2, 2, 3, 3]
BANK_LO = [0, 2, 5, 7]                   # group in partitions 0:64 of bank
BANK_HI = [1, 3, 4, 6]                   # group in partitions 64:128

_PROG_CACHE = {}


def _build_program():
    import concourse.bacc as bacc
    import concourse.mybir as mybir
    import concourse.tile as tile

    dt = mybir.dt
    nc = bacc.Bacc(
        "TRN2",
        target_bir_lowering=False,
        debug=False,
        num_devices=N_CORES,
    )

    f32 = dt.float32
    f16 = dt.float16

    xT = nc.dram_tensor("xT", [B_PER_CORE, 2, 128, XW], f16,
                        kind="ExternalInput")
    wT = nc.dram_tensor("wT", [128, 9 * 128], f16, kind="ExternalInput")
    bT = nc.dram_tensor("bT", [128, 4], f32, kind="ExternalInput")
    outT = nc.dram_tensor("outT", [B_PER_CORE, 4, 128, OW], f16,
                          kind="ExternalOutput")

    with tile.TileContext(nc) as tc:
        with (
            tc.tile_pool(name="const", bufs=1) as cpool,
            tc.tile_pool(name="xg", bufs=1) as xpool,
            tc.tile_pool(name="ot", bufs=2) as opool,
            tc.tile_pool(name="ps", bufs=1, space="PSUM") as ppool,
        ):
            wsb = cpool.tile([128, 9 * 128], f16)
            nc.sync.dma_start(wsb[:], wT[:])
            bsb = cpool.tile([128, 4], f32)
            nc.sync.dma_start(bsb[:], bT[:])

            # x chunks: chunk t covers flat cols [464t, 464t+582) of XW
            xch = {}
            for t in range(NT):
                for b in range(B_PER_CORE):
                    for hf in range(2):
                        xc = xpool.tile([128, CHW], f16,
                                        tag=f"x{b}{hf}{t}")
                        nc.sync.dma_start(
                            xc[:], xT[b, hf, :, NTW * t:NTW * t + CHW])
                        xch[b, hf, t] = xc

            for t in range(NT):
                for b in range(B_PER_CORE):
                    ps = []
                    for k in range(4):
                        pst = ppool.tile([128, NTW], f32, tag=f"ps{b}{k}")
                        ps.append(pst)
                    for tap in range(9):
                        kh, kw = divmod(tap, 3)
                        off = (WP + 1) + WP * (kh - 1) + (kw - 1)
                        for g in range(G):
                            a = ROWSTRIP[g]
                            ch = COLHALF[g]
                            nc.tensor.matmul(
                                ps[BANK[g]][F * ch:F * ch + F, :],
                                wsb[32 * a:32 * a + 32,
                                    128 * tap + F * ch:128 * tap + F * ch + F],
                                xch[b, g // 4, t][32 * a:32 * a + 32,
                                                  off:off + NTW],
                                start=(tap == 0),
                                stop=(tap == 8),
                                tile_position=(32 * a, F * ch),
                            )
                    for k in range(4):
                        ot = opool.tile([128, NTW], f16, tag=f"ot{b}{k}")
                        nc.vector.tensor_scalar_add(ot[:], ps[k][:],
                                                    bsb[:, k:k + 1])
                        nc.sync.dma_start(
                            outT[b, k, :, NTW * t:NTW * (t + 1)], ot[:])

    nc.compile()
    return nc


def _get_program():
    if "nc" not in _PROG_CACHE:
        _PROG_CACHE["nc"] = _build_program()
    return _PROG_CACHE["nc"]


def prepare_in_maps(x, kernels, bias):
    x = np.ascontiguousarray(x, dtype=np.float32)
    kernels = np.ascontiguousarray(kernels, dtype=np.float32)
    bias = np.ascontiguousarray(bias, dtype=np.float32)

    nb = x.shape[0]
    # flat padded channels-major x: [b, half, 128, XW], fp16
    xp = np.zeros((nb, 2, 128, HP, WP), np.float16)
    xc = x.transpose(0, 3, 1, 2).reshape(nb, 2, 128, H, W)
    xp[:, :, :, 1:1 + H, 1:1 + W] = xc.astype(np.float16)
    xT = np.zeros((nb, 2, 128, XW), np.float16)
    xT[:, :, :, 1:1 + SP] = xp.reshape(nb, 2, 128, SP)

    # weights [128, 9*128]: row strip a, tap, col half -> group's [32,64]
    wT = np.zeros((128, 9 * 128), np.float16)
    for g in range(G):
        a, ch = ROWSTRIP[g], COLHALF[g]
        for tap in range(9):
            kh, kw = divmod(tap, 3)
            wT[32 * a:32 * a + 32, 128 * tap + F * ch:128 * tap + F * ch + F] \
                = kernels[g, kh, kw].astype(np.float16)

    # bias [128, 4]: bank k = [bias of BANK_LO[k]; bias of BANK_HI[k]]
    bT = np.zeros((128, 4), np.float32)
    for k in range(4):
        bT[0:F, k] = bias[F * BANK_LO[k]:F * (BANK_LO[k] + 1)]
        bT[F:2 * F, k] = bias[F * BANK_HI[k]:F * (BANK_HI[k] + 1)]

    return [
        {"xT": np.ascontiguousarray(xT[i * B_PER_CORE:(i + 1) * B_PER_CORE]),
         "wT": wT, "bT": bT}
        for i in range(N_CORES)
    ]


def gather_output(results, nb):
    out = np.empty((nb, H, W, G * F), np.float32)
    for i in range(N_CORES):
        o = results[i]["outT"]  # [B_PER_CORE, 4, 128, OW] fp16
        o = o.reshape(B_PER_CORE, 4, 128, H, WP).astype(np.float32)
        o = o[:, :, :, :, 1:1 + W]          # drop padded cols
        for k in range(4):
            lo, hi = BANK_LO[k], BANK_HI[k]
            for b in range(B_PER_CORE):
                img = out[i * B_PER_CORE + b]
                img[:, :, F * lo:F * (lo + 1)] = o[b, k, 0:F].transpose(1, 2, 0)
                img[:, :, F * hi:F * (hi + 1)] = o[b, k, F:2 * F].transpose(1, 2, 0)
    return out


def kernel(x, kernels, bias):
    from concourse.bass_utils import run_bass_kernel_spmd

    nc = _get_program()
    in_maps = prepare_in_maps(x, kernels, bias)
    res = run_bass_kernel_spmd(nc, in_maps, list(range(N_CORES)))
    return gather_output(res.results, np.asarray(x).shape[0])


# revision 4
# speedup vs baseline: 1.5590x; 1.1110x over previous
"""Grouped 3x3 SAME conv on 8 Trainium2 NeuronCores.

Problem: x[16,56,56,256] NHWC, 8 groups of 32->64 channels, 3x3 SAME,
out[16,56,56,512], fp32.

Strategy (hardcoded):
  - Data-parallel over batch: core i handles images [2i, 2i+1].
  - Host-side prep: channels-major, zero-pad spatial to 58x58, flatten to
    3364 cols (+1 zero col each side -> 3366), fp16. NO tap replication.
  - On device: the 128x128 PE array is 16 independent 32x32 sub-arrays.
    A group's 32->64 matmul occupies a (32-row x 64-col) region; 8 groups
    tile the full array exactly. Per tap (kh,kw) the 8 groups' matmuls
    are issued back-to-back and stream concurrently (disjoint regions);
    the tap shift is a column offset into the flat padded image. 9 taps
    accumulate into PSUM (per-element has_written -> add). Full PE
    utilization vs 37.5% for the K=96/M=64 formulation.
  - PSUM: 4 banks per (image, spatial tile), double-buffered (8 total)
    so tile t+1's matmuls never wait on tile t's bias/copy-out.
  - Bias is added during PSUM->SBUF fp16 copy, split between the Vector
    and Scalar (activation) engines; one fused output DMA per (img,tile)
    on the GpSimd SWDGE ring; input x DMAs split across the two HWDGE
    rings (sync for img0, scalar for img1) in 2-tile chunks so the first
    matmul only waits ~2 DMAs.
"""

import numpy as np

G = 8        # groups
P = 32       # in-channels per group
F = 64       # out-channels per group
H = W = 56
HP = WP = 58           # zero-padded spatial
SP = HP * WP           # 3364 padded pixels
XW = SP + 2            # flat padded width (+1 zero col each side)
N_CORES = 8
B_PER_CORE = 2
NT = 7                 # spatial tiles (8 image rows each)
NTW = 8 * WP           # 464 output cols per tile (<=512: one PSUM bank)
OW = NT * NTW          # 3248 stored cols per image (rows 1..56)
# input chunks: (start col, width) in the XW-padded space; chunk c serves
# tiles 2c and 2c+1 (halo of 59 cols each side)
CHUNKS = [(0, 2 * NTW + 118), (2 * NTW, 2 * NTW + 118),
          (4 * NTW, 2 * NTW + 118), (6 * NTW, NTW + 118)]

# group -> (row strip, col half == psum half, bank); the 16 (row,colhalf)
# strips cover the array exactly: left col-half by strip = [0,5,2,7],
# right = [4,1,6,3]; banks: 0=[g0|g1] 1=[g2|g3] 2=[g5|g4] 3=[g7|g6]
ROWSTRIP = [0, 1, 2, 3, 0, 1, 2, 3]      # == g % 4
COLHALF = [0, 1, 0, 1, 1, 0, 1, 0]
BANK = [0, 0, 1, 1, 2, 2, 3, 3]
BANK_LO = [0, 2, 5, 7]                   # group in partitions 0:64 of bank
BANK_HI = [1, 3, 4, 6]                   # group in partitions 64:128

_PROG_CACHE = {}


def _build_program():
    import concourse.bacc as bacc
    import concourse.mybir as mybir
    import concourse.tile as tile

    dt = mybir.dt
    act = mybir.ActivationFunctionType
    nc = bacc.Bacc(
        "TRN2",
        target_bir_lowering=False,
        debug=False,
        num_devices=N_CORES,
    )

    f32 = dt.float32
    f16 = dt.float16

    xT = nc.dram_tensor("xT", [B_PER_CORE, 2, 128, XW], f16,
                        kind="ExternalInput")
    wT = nc.dram_tensor("wT", [128, 9 * 128], f16, kind="ExternalInput")
    bT = nc.dram_tensor("bT", [128, 4], f32, kind="ExternalInput")
    outT = nc.dram_tensor("outT", [B_PER_CORE, 128, 4, OW], f16,
                          kind="ExternalOutput")

    with tile.TileContext(nc) as tc:
        with (
            tc.tile_pool(name="const", bufs=1) as cpool,
            tc.tile_pool(name="xg", bufs=1) as xpool,
            tc.tile_pool(name="ot", bufs=2) as opool,
            tc.tile_pool(name="ps", bufs=2, space="PSUM") as ppool,
        ):
            wsb = cpool.tile([128, 9 * 128], f16)
            nc.scalar.dma_start(wsb[:], wT[:])
            bsb = cpool.tile([128, 4], f32)
            nc.sync.dma_start(bsb[:], bT[:])

            # x chunks: img0 on the sync HWDGE ring, img1 on scalar's
            xch = {}
            for c, (c0, cw) in enumerate(CHUNKS):
                for b in range(B_PER_CORE):
                    for hf in range(2):
                        xc = xpool.tile([128, cw], f16, tag=f"x{b}{hf}{c}")
                        eng = nc.sync if b == 0 else nc.scalar
                        eng.dma_start(xc[:], xT[b, hf, :, c0:c0 + cw])
                        xch[b, hf, c] = xc

            for b in range(B_PER_CORE):
                for t in range(NT):
                    loff = (t % 2) * NTW + WP + 1  # base col in chunk
                    ps = []
                    for k in range(4):
                        pst = ppool.tile([128, NTW], f32, tag=f"ps{k}")
                        ps.append(pst)
                    for tap in range(9):
                        kh, kw = divmod(tap, 3)
                        off = loff + WP * (kh - 1) + (kw - 1)
                        for g in range(G):
                            a = ROWSTRIP[g]
                            ch = COLHALF[g]
                            nc.tensor.matmul(
                                ps[BANK[g]][F * ch:F * ch + F, :],
                                wsb[32 * a:32 * a + 32,
                                    128 * tap + F * ch:128 * tap + F * ch + F],
                                xch[b, g // 4, t // 2][32 * a:32 * a + 32,
                                                       off:off + NTW],
                                start=(tap == 0),
                                stop=(tap == 8),
                                tile_position=(32 * a, F * ch),
                            )
                    ot = opool.tile([128, 4 * NTW], f16, tag=f"ot{b}")
                    for k in range(4):
                        if k % 2 == 0:
                            nc.vector.tensor_scalar_add(
                                ot[:, NTW * k:NTW * (k + 1)], ps[k][:],
                                bsb[:, k:k + 1])
                        else:
                            nc.scalar.activation(
                                ot[:, NTW * k:NTW * (k + 1)], ps[k][:],
                                act.Identity, bias=bsb[:, k:k + 1])
                    nc.gpsimd.dma_start(
                        outT[b, :, :, NTW * t:NTW * (t + 1)], ot[:])

    nc.compile()
    return nc


def _get_program():
    if "nc" not in _PROG_CACHE:
        _PROG_CACHE["nc"] = _build_program()
    return _PROG_CACHE["nc"]


def prepare_in_maps(x, kernels, bias):
    x = np.ascontiguousarray(x, dtype=np.float32)
    kernels = np.ascontiguousarray(kernels, dtype=np.float32)
    bias = np.ascontiguousarray(bias, dtype=np.float32)

    nb = x.shape[0]
    # flat padded channels-major x: [b, half, 128, XW], fp16
    xp = np.zeros((nb, 2, 128, HP, WP), np.float16)
    xc = x.transpose(0, 3, 1, 2).reshape(nb, 2, 128, H, W)
    xp[:, :, :, 1:1 + H, 1:1 + W] = xc.astype(np.float16)
    xT = np.zeros((nb, 2, 128, XW), np.float16)
    xT[:, :, :, 1:1 + SP] = xp.reshape(nb, 2, 128, SP)

    # weights [128, 9*128]: row strip a, tap, col half -> group's [32,64]
    wT = np.zeros((128, 9 * 128), np.float16)
    for g in range(G):
        a, ch = ROWSTRIP[g], COLHALF[g]
        for tap in range(9):
            kh, kw = divmod(tap, 3)
            wT[32 * a:32 * a + 32, 128 * tap + F * ch:128 * tap + F * ch + F] \
                = kernels[g, kh, kw].astype(np.float16)

    # bias [128, 4]: bank k = [bias of BANK_LO[k]; bias of BANK_HI[k]]
    bT = np.zeros((128, 4), np.float32)
    for k in range(4):
        bT[0:F, k] = bias[F * BANK_LO[k]:F * (BANK_LO[k] + 1)]
        bT[F:2 * F, k] = bias[F * BANK_HI[k]:F * (BANK_HI[k] + 1)]

    return [
        {"xT": np.ascontiguousarray(xT[i * B_PER_CORE:(i + 1) * B_PER_CORE]),
         "wT": wT, "bT": bT}
        for i in range(N_CORES)
    ]


def gather_output(results, nb):
    out = np.empty((nb, H, W, G * F), np.float32)
    for i in range(N_CORES):
        o = results[i]["outT"]  # [B_PER_CORE, 128, 4, OW] fp16
        o = o.transpose(0, 2, 1, 3).reshape(B_PER_CORE, 4, 128, H, WP)
        o = o.astype(np.float32)[:, :, :, :, 1:1 + W]   # drop padded cols
        for k in range(4):
            lo, hi = BANK_LO[k], BANK_HI[k]
            for b in range(B_PER_CORE):
                img = out[i * B_PER_CORE + b]
                img[:, :, F * lo:F * (lo + 1)] = o[b, k, 0:F].transpose(1, 2, 0)
                img[:, :, F * hi:F * (hi + 1)] = o[b, k, F:2 * F].transpose(1, 2, 0)
    return out


def kernel(x, kernels, bias):
    from concourse.bass_utils import run_bass_kernel_spmd

    nc = _get_program()
    in_maps = prepare_in_maps(x, kernels, bias)
    res = run_bass_kernel_spmd(nc, in_maps, list(range(N_CORES)))
    return gather_output(res.results, np.asarray(x).shape[0])


# revision 7
# speedup vs baseline: 1.8177x; 1.1659x over previous
"""Grouped 3x3 SAME conv on 8 Trainium2 NeuronCores.

Problem: x[16,56,56,256] NHWC, 8 groups of 32->64 channels, 3x3 SAME,
out[16,56,56,512], fp32.

Strategy (hardcoded):
  - Data-parallel over batch: core i handles images [2i, 2i+1].
  - Host-side prep: channels-major, zero-pad spatial to 58x58, flatten to
    3364 cols (+1 zero col each side -> 3366), fp16. NO tap replication.
  - On device: the 128x128 PE array is 16 independent 32x32 sub-arrays.
    A group's 32->64 matmul occupies a (32-row x 64-col) region; 8 groups
    tile the full array exactly. Per tap (kh,kw) the 8 groups' matmuls
    are issued back-to-back and stream concurrently (disjoint regions);
    the tap shift is a column offset into the flat padded image. 9 taps
    accumulate into PSUM (per-element has_written -> add). Full PE
    utilization vs 37.5% for the K=96/M=64 formulation.
  - PSUM: 4 banks per (image, spatial tile), double-buffered (8 total)
    so tile t+1's matmuls never wait on tile t's bias/copy-out.
  - Bias is added during PSUM->SBUF fp16 copy, split between the Vector
    and Scalar (activation) engines; one fused output DMA per (img,tile)
    on the GpSimd SWDGE ring; input x DMAs split across the two HWDGE
    rings (sync for img0, scalar for img1) in 2-tile chunks so the first
    matmul only waits ~2 DMAs.
"""

import numpy as np

G = 8        # groups
P = 32       # in-channels per group
F = 64       # out-channels per group
H = W = 56
HP = WP = 58           # zero-padded spatial
SP = HP * WP           # 3364 padded pixels
XW = SP + 2            # flat padded width (+1 zero col each side)
N_CORES = 8
B_PER_CORE = 2
NT = 7                 # spatial tiles (8 image rows each)
NTW = 8 * WP           # 464 output cols per tile (<=512: one PSUM bank)
OW = NT * NTW          # 3248 stored cols per image (rows 1..56)
# input chunks: (start col, width) in the XW-padded space; chunk c serves
# tiles 2c and 2c+1 (halo of 59 cols each side)
CHUNKS = [(0, 2 * NTW + 118), (2 * NTW, 2 * NTW + 118),
          (4 * NTW, 2 * NTW + 118), (6 * NTW, NTW + 118)]

# group -> (row strip, col half == psum half, bank); the 16 (row,colhalf)
# strips cover the array exactly: left col-half by strip = [0,5,2,7],
# right = [4,1,6,3]; banks: 0=[g0|g1] 1=[g2|g3] 2=[g5|g4] 3=[g7|g6]
ROWSTRIP = [0, 1, 2, 3, 0, 1, 2, 3]      # == g % 4
COLHALF = [0, 1, 0, 1, 1, 0, 1, 0]
BANK = [0, 0, 1, 1, 2, 2, 3, 3]
BANK_LO = [0, 2, 5, 7]                   # group in partitions 0:64 of bank
BANK_HI = [1, 3, 4, 6]                   # group in partitions 64:128

_PROG_CACHE = {}


def _build_program():
    import concourse.bacc as bacc
    import concourse.mybir as mybir
    import concourse.tile as tile

    dt = mybir.dt
    act = mybir.ActivationFunctionType
    nc = bacc.Bacc(
        "TRN2",
        target_bir_lowering=False,
        debug=False,
        num_devices=N_CORES,
    )

    f32 = dt.float32
    f16 = dt.float16

    xT = nc.dram_tensor("xT", [B_PER_CORE, 2, 128, XW], f16,
                        kind="ExternalInput")
    wT = nc.dram_tensor("wT", [128, 9 * 128], f16, kind="ExternalInput")
    bT = nc.dram_tensor("bT", [128, 4], f32, kind="ExternalInput")
    outT = nc.dram_tensor("outT", [B_PER_CORE, 128, 4, OW], f16,
                          kind="ExternalOutput")

    with tile.TileContext(nc) as tc:
        with (
            tc.tile_pool(name="const", bufs=1) as cpool,
            tc.tile_pool(name="xg", bufs=1) as xpool,
            tc.tile_pool(name="ot", bufs=3) as opool,
            tc.tile_pool(name="ps", bufs=2, space="PSUM") as ppool,
        ):
            wsb = cpool.tile([128, 9 * 128], f16)
            nc.scalar.dma_start(wsb[:], wT[:])
            bsb = cpool.tile([128, 4], f32)

            # x chunks: img0 on the sync HWDGE ring, img1 on scalar's;
            # the first tile's chunks lead both rings so MMs start early
            xch = {}
            for c, (c0, cw) in enumerate(CHUNKS):
                for b in range(B_PER_CORE):
                    for hf in range(2):
                        xc = xpool.tile([128, cw], f16, tag=f"x{b}{hf}{c}")
                        eng = nc.sync if b == 0 else nc.scalar
                        eng.dma_start(xc[:], xT[b, hf, :, c0:c0 + cw])
                        xch[b, hf, c] = xc
                if c == 0:
                    nc.sync.dma_start(bsb[:], bT[:])

            for b in range(B_PER_CORE):
                for t in range(NT):
                    loff = (t % 2) * NTW + WP + 1  # base col in chunk
                    ps = []
                    for k in range(4):
                        pst = ppool.tile([128, NTW], f32, tag=f"ps{k}")
                        ps.append(pst)
                    for tap in range(9):
                        kh, kw = divmod(tap, 3)
                        off = loff + WP * (kh - 1) + (kw - 1)
                        for g in range(G):
                            a = ROWSTRIP[g]
                            ch = COLHALF[g]
                            nc.tensor.matmul(
                                ps[BANK[g]][F * ch:F * ch + F, :],
                                wsb[32 * a:32 * a + 32,
                                    128 * tap + F * ch:128 * tap + F * ch + F],
                                xch[b, g // 4, t // 2][32 * a:32 * a + 32,
                                                       off:off + NTW],
                                start=(tap == 0),
                                stop=(tap == 8),
                                tile_position=(32 * a, F * ch),
                            )
                    ot = opool.tile([128, 4 * NTW], f16, tag=f"ot{b}")
                    for k in range(4):
                        if k % 2 == 0:
                            nc.vector.tensor_scalar_add(
                                ot[:, NTW * k:NTW * (k + 1)], ps[k][:],
                                bsb[:, k:k + 1])
                        else:
                            nc.scalar.activation(
                                ot[:, NTW * k:NTW * (k + 1)], ps[k][:],
                                act.Identity, bias=bsb[:, k:k + 1])
                    nc.sync.dma_start(
                        outT[b, :, :, NTW * t:NTW * (t + 1)], ot[:])

    nc.compile()
    return nc


def _get_program():
    if "nc" not in _PROG_CACHE:
        _PROG_CACHE["nc"] = _build_program()
    return _PROG_CACHE["nc"]


def prepare_in_maps(x, kernels, bias):
    x = np.ascontiguousarray(x, dtype=np.float32)
    kernels = np.ascontiguousarray(kernels, dtype=np.float32)
    bias = np.ascontiguousarray(bias, dtype=np.float32)

    nb = x.shape[0]
    # flat padded channels-major x: [b, half, 128, XW], fp16
    xp = np.zeros((nb, 2, 128, HP, WP), np.float16)
    xc = x.transpose(0, 3, 1, 2).reshape(nb, 2, 128, H, W)
    xp[:, :, :, 1:1 + H, 1:1 + W] = xc.astype(np.float16)
    xT = np.zeros((nb, 2, 128, XW), np.float16)
    xT[:, :, :, 1:1 + SP] = xp.reshape(nb, 2, 128, SP)

    # weights [128, 9*128]: row strip a, tap, col half -> group's [32,64]
    wT = np.zeros((128, 9 * 128), np.float16)
    for g in range(G):
        a, ch = ROWSTRIP[g], COLHALF[g]
        for tap in range(9):
            kh, kw = divmod(tap, 3)
            wT[32 * a:32 * a + 32, 128 * tap + F * ch:128 * tap + F * ch + F] \
                = kernels[g, kh, kw].astype(np.float16)

    # bias [128, 4]: bank k = [bias of BANK_LO[k]; bias of BANK_HI[k]]
    bT = np.zeros((128, 4), np.float32)
    for k in range(4):
        bT[0:F, k] = bias[F * BANK_LO[k]:F * (BANK_LO[k] + 1)]
        bT[F:2 * F, k] = bias[F * BANK_HI[k]:F * (BANK_HI[k] + 1)]

    return [
        {"xT": np.ascontiguousarray(xT[i * B_PER_CORE:(i + 1) * B_PER_CORE]),
         "wT": wT, "bT": bT}
        for i in range(N_CORES)
    ]


def gather_output(results, nb):
    out = np.empty((nb, H, W, G * F), np.float32)
    for i in range(N_CORES):
        o = results[i]["outT"]  # [B_PER_CORE, 128, 4, OW] fp16
        o = o.transpose(0, 2, 1, 3).reshape(B_PER_CORE, 4, 128, H, WP)
        o = o.astype(np.float32)[:, :, :, :, 1:1 + W]   # drop padded cols
        for k in range(4):
            lo, hi = BANK_LO[k], BANK_HI[k]
            for b in range(B_PER_CORE):
                img = out[i * B_PER_CORE + b]
                img[:, :, F * lo:F * (lo + 1)] = o[b, k, 0:F].transpose(1, 2, 0)
                img[:, :, F * hi:F * (hi + 1)] = o[b, k, F:2 * F].transpose(1, 2, 0)
    return out


def kernel(x, kernels, bias):
    from concourse.bass_utils import run_bass_kernel_spmd

    nc = _get_program()
    in_maps = prepare_in_maps(x, kernels, bias)
    res = run_bass_kernel_spmd(nc, in_maps, list(range(N_CORES)))
    return gather_output(res.results, np.asarray(x).shape[0])
